# revision 1
# baseline (speedup 1.0000x reference)
"""GRU decoder kernel for Trainium2 (8 NeuronCores, data-parallel over batch).

Design:
 - Shard B=512 across 8 cores (64 per core); replicate all weights.
 - Per core, the 3-layer GRU scan runs as a layer-staggered wavefront:
   at tick tau, cell0 computes h0[tau], cell1 computes h1[tau-1], cell2
   computes h2[tau-2].  All 5 matmul groups of a tick depend only on
   state produced in earlier ticks, so PE never stalls on the EW chain.
 - Matmuls are batch-stationary: lhsT = h^T (hidden-major, [128 K-chunk, 64]),
   moving operand = W^T chunks [128, <=512] in float32r (full-rate), psum out
   batch-major [64, gates].  Biases ride on a ones-row of h^T (hidden row 501).
 - cell1 uses psum partition rows 0:64, cell2 rows 64:128 of shared banks
   (2-way PE column tiling -> concurrent matmul streams).
 - gi (input-side) matmuls accumulate onto gh's psum for the r,z gates;
   the n-gate gi goes to a separate psum bank (PyTorch GRU semantics).
 - gi0 (constant across time) is accumulated into cell0's psum each step
   with a cheap identity matmul.
 - fc1+selu+gi0 prologue and fc2+softmax epilogue run on-chip as well.
"""

import os
import sys

sys.path.insert(0, "/opt/trn_rl_repo")

import numpy as np

import concourse.bass as bass
import concourse.mybir as mybir
import concourse.tile as tile
from concourse import bacc
from concourse import bass_utils
from concourse.masks import make_identity

F32 = mybir.dt.float32
F32R = mybir.dt.float32r
AX = mybir.AxisListType
ALU = mybir.AluOpType
ACTF = mybir.ActivationFunctionType

D_LATENT = 292
D_CHAR = 35
H = 501
G = 3 * H  # 1503
GP = 1504  # padded gate dim (even matmul chunk widths)
CP = 36    # padded char dim
T = int(os.environ.get("BASS_GRU_T", "120"))
BATCH = 512
NCORES = 8
BC = BATCH // NCORES  # 64 per core

SELU_L = 1.0507009873554804934193349852946
SELU_A = 1.6732632423543772848170429916717

# gh matmul N-chunks (bank-aligned)
NCH = [(0, 512), (512, 512), (1024, 480)]
# gi matmul N-chunks: r,z accumulate into gh psum; n goes to its own bank
GICH_RZ = [(0, 512), (512, 490)]
GI_N = (1002, 502)
# K chunking of padded hidden (512 = 4*128), ones row at 501
KC = 4
ONES_ROW = 501  # = 3*128 + 117
# K chunking of padded latent (384 = 3*128), ones row at 292
KCX = 3
ONES_ROW_X = 292  # = 2*128 + 36

_CACHE = {}


def _mmr(nc, out, lhsT, rhs, start, stop):
    nc.tensor.matmul(out, lhsT, rhs, start=start, stop=stop)


def build_bass():
    nc = bacc.Bacc("TRN2", target_bir_lowering=False, debug=False)

    # ---- DRAM I/O ----
    z_in = nc.dram_tensor("z_in", [BC, D_LATENT], F32, kind="ExternalInput").ap()
    w1s_d = nc.dram_tensor("w1s", [128, KCX, D_LATENT], F32R, kind="ExternalInput").ap()
    wih0_d = nc.dram_tensor("wih0s", [128, KCX, GP], F32R, kind="ExternalInput").ap()
    whh0_d = nc.dram_tensor("whh0s", [128, KC, GP], F32R, kind="ExternalInput").ap()
    wih1_d = nc.dram_tensor("wih1s", [128, KC, GP], F32R, kind="ExternalInput").ap()
    whh1_d = nc.dram_tensor("whh1s", [128, KC, GP], F32R, kind="ExternalInput").ap()
    wih2_d = nc.dram_tensor("wih2s", [128, KC, GP], F32R, kind="ExternalInput").ap()
    whh2_d = nc.dram_tensor("whh2s", [128, KC, GP], F32R, kind="ExternalInput").ap()
    w2s_d = nc.dram_tensor("w2s", [128, KC, CP], F32R, kind="ExternalInput").ap()
    id64_d = nc.dram_tensor("id64_in", [64, 64], F32R, kind="ExternalInput").ap()
    htini_d = nc.dram_tensor("ht_init", [128, KC, BC], F32R, kind="ExternalInput").ap()
    xtini_d = nc.dram_tensor("xt_init", [128, KCX, BC], F32R, kind="ExternalInput").ap()
    probs = nc.dram_tensor("probs", [BC, T, D_CHAR], F32, kind="ExternalOutput").ap()
    h2t_d = nc.dram_tensor("h2t_scratch", [T, 128, KC * BC], F32R).ap()

    with tile.TileContext(nc) as tc:
        with tc.tile_pool(name="singles", bufs=1) as sg:
            # ---- load weights ----
            w1s = sg.tile([128, KCX, D_LATENT], F32R)
            wih0 = sg.tile([128, KCX, GP], F32R)
            whh0 = sg.tile([128, KC, GP], F32R)
            wih1 = sg.tile([128, KC, GP], F32R)
            whh1 = sg.tile([128, KC, GP], F32R)
            wih2 = sg.tile([128, KC, GP], F32R)
            whh2 = sg.tile([128, KC, GP], F32R)
            w2s = sg.tile([128, KC, CP], F32R)
            for dst, src in [
                (w1s, w1s_d), (wih0, wih0_d), (whh0, whh0_d), (wih1, wih1_d),
                (whh1, whh1_d), (wih2, wih2_d), (whh2, whh2_d), (w2s, w2s_d),
            ]:
                nc.sync.dma_start(out=dst, in_=src)

            ident = sg.tile([128, 128], F32)
            make_identity(nc, ident)
            id64 = ident[0:64, 0:64]
            id64r = sg.tile([64, 64], F32R)
            nc.sync.dma_start(out=id64r, in_=id64_d)

            # persistent state
            h0T = sg.tile([128, KC, BC], F32R)
            h1T = sg.tile([128, KC, BC], F32R)
            h2T = sg.tile([128, KC, BC], F32R)
            h0b = sg.tile([BC, H], F32)
            h1b = sg.tile([BC, H], F32)
            h2b = sg.tile([BC, H], F32)
            gi0 = sg.tile([BC, 1536], F32R)
            gi0n = sg.tile([BC, H], F32)
            for t_ in (h0T, h1T, h2T):
                nc.sync.dma_start(out=t_, in_=htini_d)  # zeros + ones row (idx 501)
            for t_ in (h0b, h1b, h2b):
                nc.vector.memset(t_, 0.0)

            # ================= prologue: x = selu(fc1(z)); gi0 = x @ wih0 =========
            with tc.tile_pool(name="ppsum", bufs=1, space="PSUM") as pp, \
                 tc.tile_pool(name="ptmp", bufs=1) as pt:
                zsb = pt.tile([BC, D_LATENT], F32)
                nc.sync.dma_start(out=zsb, in_=z_in)
                trp = pp.tile([128, KCX, BC], F32)
                zT = pt.tile([128, KCX, BC], F32R)
                uT = pt.tile([128, KCX, BC], F32R)
                for t_ in (zT, uT):
                    nc.sync.dma_start(out=t_, in_=xtini_d)  # zeros + ones row (idx 292)
                chx = [(0, 128), (1, 128), (2, 36)]
                for c, w in chx:
                    nc.tensor.transpose(trp[0:w, c, :], zsb[:, c * 128:c * 128 + w], id64)
                    nc.scalar.copy(out=zT[0:w, c, :], in_=trp[0:w, c, :])
                xp = pp.tile([BC, D_LATENT], F32)
                for c in range(KCX):
                    _mmr(nc, xp, zT[:, c, :], w1s[:, c, :], c == 0, c == KCX - 1)
                # selu (scale folded into wih0): u = relu(x) + min(0, a*(exp(x)-1))
                esb = pt.tile([BC, D_LATENT], F32)
                nc.scalar.activation(esb, xp, ACTF.Exp)
                t1 = pt.tile([BC, D_LATENT], F32)
                nc.vector.tensor_scalar(
                    out=t1, in0=esb, scalar1=1.0, scalar2=SELU_A,
                    op0=ALU.subtract, op1=ALU.mult)
                t2 = pt.tile([BC, D_LATENT], F32)
                nc.vector.tensor_scalar(
                    out=t2, in0=t1, scalar1=0.0, scalar2=0.0,
                    op0=ALU.min, op1=ALU.bypass)
                usb = pt.tile([BC, D_LATENT], F32)
                nc.vector.scalar_tensor_tensor(
                    out=usb, in0=xp, scalar=0.0, in1=t2,
                    op0=ALU.max, op1=ALU.add)
                for c, w in chx:
                    nc.tensor.transpose(trp[0:w, c, :], usb[:, c * 128:c * 128 + w], id64)
                    nc.scalar.copy(out=uT[0:w, c, :], in_=trp[0:w, c, :])
                g0p = pp.tile([BC, 1536], F32)
                for c in range(KCX):
                    for lo, w in NCH:
                        _mmr(nc, g0p[:, lo:lo + w], uT[:, c, :], wih0[:, c, lo:lo + w],
                             c == 0, c == KCX - 1)
                for lo, w in NCH:
                    nc.scalar.copy(out=gi0[:, lo:lo + w], in_=g0p[:, lo:lo + w])
                nc.scalar.copy(out=gi0n, in_=g0p[:, 2 * H:3 * H])

            # ================= scan: layer-staggered wavefront =================
            with tc.tile_pool(name="spsum", bufs=1, space="PSUM") as sp, \
                 tc.tile_pool(name="wk", bufs=2) as wk:
                pghA = sp.tile([BC, 1536], F32)  # cell1, then cell0 (time-shared)
                pghB = sp.tile([BC, 1536], F32)  # cell2
                pgin = sp.tile([BC, 512], F32)   # gi1_n then gi2_n (time-shared)
                ptr = sp.tile([128, KC, BC], F32)

                chh = [(0, 128), (1, 128), (2, 128), (3, 117)]

                def gh_gi_mms(pgh, hgT_prev, hgi_in, whh, wih):
                    for lo, w in NCH:
                        for c in range(KC):
                            _mmr(nc, pgh[:, lo:lo + w], hgT_prev[:, c, :],
                                 whh[:, c, lo:lo + w], c == 0,
                                 (lo == 1024 and c == KC - 1))
                    for lo, w in GICH_RZ:
                        for c in range(KC):
                            _mmr(nc, pgh[:, lo:lo + w], hgi_in[:, c, :],
                                 wih[:, c, lo:lo + w], False, c == KC - 1)
                    lo, w = GI_N
                    for c in range(KC):
                        _mmr(nc, pgin[:, 0:w], hgi_in[:, c, :],
                             wih[:, c, lo:lo + w], c == 0, c == KC - 1)

                def ew_cell(pgh, pginap, gin_sb, hb, hT, dma_t):
                    r = wk.tile([BC, H], F32, tag="r")
                    z = wk.tile([BC, H], F32, tag="z")
                    nc.scalar.activation(r, pgh[:, 0:H], ACTF.Sigmoid)
                    nc.scalar.activation(z, pgh[:, H:2 * H], ACTF.Sigmoid)
                    tmp = wk.tile([BC, H], F32, tag="tmp")
                    nc.vector.tensor_mul(tmp, r, pgh[:, 2 * H:3 * H])
                    s = wk.tile([BC, H], F32, tag="s")
                    if gin_sb is not None:
                        nc.vector.tensor_add(s, tmp, gin_sb)
                    else:
                        nc.vector.tensor_add(s, tmp, pginap)
                    n = wk.tile([BC, H], F32, tag="n")
                    nc.scalar.activation(n, s, ACTF.Tanh)
                    pre = wk.tile([BC, H], F32, tag="pre")
                    nc.gpsimd.tensor_mul(pre, z, hb)  # z*h
                    m = wk.tile([BC, H], F32, tag="m")
                    nc.vector.scalar_tensor_tensor(
                        out=m, in0=z, scalar=1.0, in1=n,
                        op0=ALU.subtract, op1=ALU.mult)  # (z-1)*n
                    nc.gpsimd.tensor_sub(hb, pre, m)  # h' = z*h + (1-z)*n
                    for c, w in chh:
                        nc.tensor.transpose(ptr[0:w, c, :], hb[:, c * 128:c * 128 + w], id64)
                        if c % 2 == 0:
                            nc.scalar.copy(out=hT[0:w, c, :], in_=ptr[0:w, c, :])
                        else:
                            nc.vector.tensor_copy(hT[0:w, c, :], ptr[0:w, c, :])
                    if dma_t is not None:
                        nc.sync.dma_start(
                            out=h2t_d[dma_t],
                            in_=hT.rearrange("p c b -> p (c b)"))

                for tau in range(T + 2):
                    do0 = tau < T
                    do1 = 0 <= tau - 1 < T
                    do2 = 0 <= tau - 2 < T
                    # order: cell2, E2, cell1, E1, cell0, E0 so each pgin/pghA
                    # read follows its own writer in program order, while each
                    # cell's EW chain overlaps the next cell's matmuls on PE.
                    if do2:
                        gh_gi_mms(pghB, h2T, h1T, whh2, wih2)
                        ew_cell(pghB, pgin[:, 0:H], None, h2b, h2T, tau - 2)
                    if do1:
                        gh_gi_mms(pghA, h1T, h0T, whh1, wih1)
                        ew_cell(pghA, pgin[:, 0:H], None, h1b, h1T, None)
                    if do0:
                        for lo, w in NCH:
                            for c in range(KC):
                                _mmr(nc, pghA[:, lo:lo + w], h0T[:, c, :],
                                     whh0[:, c, lo:lo + w], c == 0,
                                     (lo == 1024 and c == KC - 1))
                        for lo, w in GICH_RZ:
                            _mmr(nc, pghA[:, lo:lo + w], id64r, gi0[:, lo:lo + w],
                                 False, True)
                        ew_cell(pghA, None, gi0n, h0b, h0T, None)

            # ================= epilogue: fc2 + softmax =================
            with tc.tile_pool(name="fpsum", bufs=2, space="PSUM") as fp, \
                 tc.tile_pool(name="fwk", bufs=3) as fw:
                for t in range(T):
                    ht = fw.tile([128, KC, BC], F32R, tag="ht")
                    nc.sync.dma_start(
                        out=ht.rearrange("p c b -> p (c b)"), in_=h2t_d[t])
                    pf = fp.tile([BC, CP], F32, tag="pf")
                    for c in range(KC):
                        _mmr(nc, pf, ht[:, c, :], w2s[:, c, :], c == 0, c == KC - 1)
                    e = fw.tile([BC, D_CHAR], F32, tag="e")
                    nc.scalar.activation(e, pf[:, 0:D_CHAR], ACTF.Exp)
                    ssum = fw.tile([BC, 1], F32, tag="ssum")
                    nc.vector.reduce_sum(ssum, e, axis=AX.X)
                    rcp = fw.tile([BC, 1], F32, tag="rcp")
                    nc.vector.reciprocal(rcp, ssum)
                    pb = fw.tile([BC, D_CHAR], F32, tag="pb")
                    nc.vector.tensor_scalar_mul(pb, in0=e, scalar1=rcp)
                    nc.sync.dma_start(out=probs[:, t, :], in_=pb)

    nc.compile()
    return nc


def _prep_rec(w, b, kc, ones_row):
    """weight [Gout, Kin] + bias -> [128, kc, Gout_padded] with bias on ones_row."""
    gout, kin = w.shape
    gpad = gout + (gout % 2)
    arr = np.zeros((128, kc, gpad), dtype=np.float32)
    wt = np.ascontiguousarray(w.T)  # [Kin, Gout]
    for c in range(kc):
        lo = c * 128
        hi = min(lo + 128, kin)
        if hi > lo:
            arr[0:hi - lo, c, 0:gout] = wt[lo:hi]
    c, p = divmod(ones_row, 128)
    arr[p, c, 0:gout] = b
    return arr


def kernel(**inputs):
    inputs = {k: np.asarray(v, dtype=np.float32) for k, v in inputs.items()}
    if "nc" not in _CACHE:
        _CACHE["nc"] = build_bass()
    nc = _CACHE["nc"]

    ht_init = np.zeros((128, KC, BC), dtype=np.float32)
    ht_init[117, 3, :] = 1.0
    xt_init = np.zeros((128, KCX, BC), dtype=np.float32)
    xt_init[36, 2, :] = 1.0
    shared = {
        "id64_in": np.eye(64, dtype=np.float32),
        "ht_init": ht_init,
        "xt_init": xt_init,
        "w1s": _prep_rec(inputs["fc1_w"], inputs["fc1_b"], KCX, ONES_ROW_X),
        "wih0s": _prep_rec(SELU_L * inputs["w_ih0"], inputs["b_ih0"], KCX, ONES_ROW_X),
        "whh0s": _prep_rec(inputs["w_hh0"], inputs["b_hh0"], KC, ONES_ROW),
        "wih1s": _prep_rec(inputs["w_ih1"], inputs["b_ih1"], KC, ONES_ROW),
        "whh1s": _prep_rec(inputs["w_hh1"], inputs["b_hh1"], KC, ONES_ROW),
        "wih2s": _prep_rec(inputs["w_ih2"], inputs["b_ih2"], KC, ONES_ROW),
        "whh2s": _prep_rec(inputs["w_hh2"], inputs["b_hh2"], KC, ONES_ROW),
        "w2s": _prep_rec(inputs["fc2_w"], inputs["fc2_b"], KC, ONES_ROW),
    }
    in_maps = []
    for i in range(NCORES):
        m = dict(shared)
        m["z_in"] = np.ascontiguousarray(inputs["z"][i * BC:(i + 1) * BC])
        in_maps.append(m)

    res = bass_utils.run_bass_kernel_spmd(nc, in_maps, list(range(NCORES)))
    out = np.concatenate([r["probs"] for r in res.results], axis=0)
    return out


if __name__ == "__main__":
    np.random.seed(0)
    pass



# revision 2
# speedup vs baseline: 1748.6120x; 1748.6120x over previous
"""GRU decoder kernel for Trainium2 (8 NeuronCores, data-parallel over batch).

Design:
 - Shard B=512 across 8 cores (64 per core); replicate all weights.
 - Per core, the 3-layer GRU scan runs as a layer-staggered wavefront:
   at tick tau, cell0 computes h0[tau], cell1 computes h1[tau-1], cell2
   computes h2[tau-2].  All 5 matmul groups of a tick depend only on
   state produced in earlier ticks, so PE never stalls on the EW chain.
 - Matmuls are batch-stationary: lhsT = h^T (hidden-major, [128 K-chunk, 64]),
   moving operand = W^T chunks [128, <=512] in float32r (full-rate), psum out
   batch-major [64, gates].  Biases ride on a ones-row of h^T (hidden row 501).
 - cell1 uses psum partition rows 0:64, cell2 rows 64:128 of shared banks
   (2-way PE column tiling -> concurrent matmul streams).
 - gi (input-side) matmuls accumulate onto gh's psum for the r,z gates;
   the n-gate gi goes to a separate psum bank (PyTorch GRU semantics).
 - gi0 (constant across time) is accumulated into cell0's psum each step
   with a cheap identity matmul.
 - fc1+selu+gi0 prologue and fc2+softmax epilogue run on-chip as well.
"""

import os
import sys

sys.path.insert(0, "/opt/trn_rl_repo")

import numpy as np

import concourse.bass as bass
import concourse.mybir as mybir
import concourse.tile as tile
from concourse import bacc
from concourse import bass_utils
from concourse.masks import make_identity

F32 = mybir.dt.float32
F32R = mybir.dt.float32r
AX = mybir.AxisListType
ALU = mybir.AluOpType
ACTF = mybir.ActivationFunctionType

D_LATENT = 292
D_CHAR = 35
H = 501
G = 3 * H  # 1503
GP = 1504  # padded gate dim (even matmul chunk widths)
CP = 36    # padded char dim
T = int(os.environ.get("BASS_GRU_T", "120"))
BATCH = 512
NCORES = 8
BC = BATCH // NCORES  # 64 per core

SELU_L = 1.0507009873554804934193349852946
SELU_A = 1.6732632423543772848170429916717

# gh matmul N-chunks (bank-aligned)
NCH = [(0, 512), (512, 512), (1024, 480)]
# gi matmul N-chunks: r,z accumulate into gh psum; n goes to its own bank
GICH_RZ = [(0, 512), (512, 490)]
GI_N = (1002, 502)
# K chunking of padded hidden (512 = 4*128), ones row at 501
KC = 4
ONES_ROW = 501  # = 3*128 + 117
# K chunking of padded latent (384 = 3*128), ones row at 292
KCX = 3
ONES_ROW_X = 292  # = 2*128 + 36

_CACHE = {}


def _mmr(nc, out, lhsT, rhs, start, stop):
    nc.tensor.matmul(out, lhsT, rhs, start=start, stop=stop)


def build_bass():
    nc = bacc.Bacc("TRN2", target_bir_lowering=False, debug=False)

    # ---- DRAM I/O ----
    z_in = nc.dram_tensor("z_in", [BC, D_LATENT], F32, kind="ExternalInput").ap()
    w1s_d = nc.dram_tensor("w1s", [128, KCX, D_LATENT], F32R, kind="ExternalInput").ap()
    wih0_d = nc.dram_tensor("wih0s", [128, KCX, GP], F32R, kind="ExternalInput").ap()
    whh0_d = nc.dram_tensor("whh0s", [128, KC, GP], F32R, kind="ExternalInput").ap()
    wih1_d = nc.dram_tensor("wih1s", [128, KC, GP], F32R, kind="ExternalInput").ap()
    whh1_d = nc.dram_tensor("whh1s", [128, KC, GP], F32R, kind="ExternalInput").ap()
    wih2_d = nc.dram_tensor("wih2s", [128, KC, GP], F32R, kind="ExternalInput").ap()
    whh2_d = nc.dram_tensor("whh2s", [128, KC, GP], F32R, kind="ExternalInput").ap()
    w2s_d = nc.dram_tensor("w2s", [128, KC, CP], F32R, kind="ExternalInput").ap()
    id64_d = nc.dram_tensor("id64_in", [64, 64], F32R, kind="ExternalInput").ap()
    htini_d = nc.dram_tensor("ht_init", [128, KC, BC], F32R, kind="ExternalInput").ap()
    xtini_d = nc.dram_tensor("xt_init", [128, KCX, BC], F32R, kind="ExternalInput").ap()
    probs = nc.dram_tensor("probs", [BC, T, D_CHAR], F32, kind="ExternalOutput").ap()
    h2t_d = nc.dram_tensor("h2t_scratch", [T, 128, KC * BC], F32R).ap()

    with tile.TileContext(nc) as tc:
        with tc.tile_pool(name="singles", bufs=1) as sg:
            # ---- load weights ----
            w1s = sg.tile([128, KCX, D_LATENT], F32R)
            wih0 = sg.tile([128, KCX, GP], F32R)
            whh0 = sg.tile([128, KC, GP], F32R)
            wih1 = sg.tile([128, KC, GP], F32R)
            whh1 = sg.tile([128, KC, GP], F32R)
            wih2 = sg.tile([128, KC, GP], F32R)
            whh2 = sg.tile([128, KC, GP], F32R)
            w2s = sg.tile([128, KC, CP], F32R)
            for dst, src in [
                (w1s, w1s_d), (wih0, wih0_d), (whh0, whh0_d), (wih1, wih1_d),
                (whh1, whh1_d), (wih2, wih2_d), (whh2, whh2_d), (w2s, w2s_d),
            ]:
                nc.sync.dma_start(out=dst, in_=src)

            ident = sg.tile([128, 128], F32)
            make_identity(nc, ident)
            id64 = ident[0:64, 0:64]
            id64r = sg.tile([64, 64], F32R)
            nc.sync.dma_start(out=id64r, in_=id64_d)

            # persistent state
            h0T = sg.tile([128, KC, BC], F32R)
            h1T = sg.tile([128, KC, BC], F32R)
            h2T = sg.tile([128, KC, BC], F32R)
            h0b = sg.tile([BC, H], F32)
            h1b = sg.tile([BC, H], F32)
            h2b = sg.tile([BC, H], F32)
            gi0 = sg.tile([BC, 1536], F32R)
            gi0n = sg.tile([BC, H], F32)
            for t_ in (h0T, h1T, h2T):
                nc.sync.dma_start(out=t_, in_=htini_d)  # zeros + ones row (idx 501)
            for t_ in (h0b, h1b, h2b):
                nc.vector.memset(t_, 0.0)

            # ================= prologue: x = selu(fc1(z)); gi0 = x @ wih0 =========
            with tc.tile_pool(name="ppsum", bufs=1, space="PSUM") as pp, \
                 tc.tile_pool(name="ptmp", bufs=1) as pt:
                zsb = pt.tile([BC, D_LATENT], F32)
                nc.sync.dma_start(out=zsb, in_=z_in)
                trp = pp.tile([128, KCX, BC], F32)
                zT = pt.tile([128, KCX, BC], F32R)
                uT = pt.tile([128, KCX, BC], F32R)
                for t_ in (zT, uT):
                    nc.sync.dma_start(out=t_, in_=xtini_d)  # zeros + ones row (idx 292)
                chx = [(0, 128), (1, 128), (2, 36)]
                for c, w in chx:
                    nc.tensor.transpose(trp[0:w, c, :], zsb[:, c * 128:c * 128 + w], id64)
                    nc.scalar.copy(out=zT[0:w, c, :], in_=trp[0:w, c, :])
                xp = pp.tile([BC, D_LATENT], F32)
                for c in range(KCX):
                    _mmr(nc, xp, zT[:, c, :], w1s[:, c, :], c == 0, c == KCX - 1)
                # selu (scale folded into wih0): u = relu(x) + min(0, a*(exp(x)-1))
                esb = pt.tile([BC, D_LATENT], F32)
                nc.scalar.activation(esb, xp, ACTF.Exp)
                t1 = pt.tile([BC, D_LATENT], F32)
                nc.vector.tensor_scalar(
                    out=t1, in0=esb, scalar1=1.0, scalar2=SELU_A,
                    op0=ALU.subtract, op1=ALU.mult)
                t2 = pt.tile([BC, D_LATENT], F32)
                nc.vector.tensor_scalar(
                    out=t2, in0=t1, scalar1=0.0, scalar2=0.0,
                    op0=ALU.min, op1=ALU.bypass)
                usb = pt.tile([BC, D_LATENT], F32)
                nc.vector.scalar_tensor_tensor(
                    out=usb, in0=xp, scalar=0.0, in1=t2,
                    op0=ALU.max, op1=ALU.add)
                for c, w in chx:
                    nc.tensor.transpose(trp[0:w, c, :], usb[:, c * 128:c * 128 + w], id64)
                    nc.scalar.copy(out=uT[0:w, c, :], in_=trp[0:w, c, :])
                g0p = pp.tile([BC, 1536], F32)
                for c in range(KCX):
                    for lo, w in NCH:
                        _mmr(nc, g0p[:, lo:lo + w], uT[:, c, :], wih0[:, c, lo:lo + w],
                             c == 0, c == KCX - 1)
                for lo, w in NCH:
                    nc.scalar.copy(out=gi0[:, lo:lo + w], in_=g0p[:, lo:lo + w])
                nc.scalar.copy(out=gi0n, in_=g0p[:, 2 * H:3 * H])

            # ================= scan: layer-staggered wavefront =================
            with tc.tile_pool(name="spsum", bufs=1, space="PSUM") as sp, \
                 tc.tile_pool(name="wk", bufs=2) as wk:
                pghA = sp.tile([BC, 1536], F32)  # cell1, then cell0 (time-shared)
                pghB = sp.tile([BC, 1536], F32)  # cell2
                pgin = sp.tile([BC, 512], F32)   # gi1_n then gi2_n (time-shared)
                ptr = sp.tile([128, KC, BC], F32)

                chh = [(0, 128), (1, 128), (2, 128), (3, 117)]

                def gh_gi_mms(pgh, hgT_prev, hgi_in, whh, wih):
                    for lo, w in NCH:
                        for c in range(KC):
                            _mmr(nc, pgh[:, lo:lo + w], hgT_prev[:, c, :],
                                 whh[:, c, lo:lo + w], c == 0,
                                 (lo == 1024 and c == KC - 1))
                    for lo, w in GICH_RZ:
                        for c in range(KC):
                            _mmr(nc, pgh[:, lo:lo + w], hgi_in[:, c, :],
                                 wih[:, c, lo:lo + w], False, c == KC - 1)
                    lo, w = GI_N
                    for c in range(KC):
                        _mmr(nc, pgin[:, 0:w], hgi_in[:, c, :],
                             wih[:, c, lo:lo + w], c == 0, c == KC - 1)

                def ew_cell(pgh, pginap, gin_sb, hb, hT, dma_t):
                    r = wk.tile([BC, H], F32, tag="r")
                    z = wk.tile([BC, H], F32, tag="z")
                    nc.scalar.activation(r, pgh[:, 0:H], ACTF.Sigmoid)
                    nc.scalar.activation(z, pgh[:, H:2 * H], ACTF.Sigmoid)
                    tmp = wk.tile([BC, H], F32, tag="tmp")
                    nc.vector.tensor_mul(tmp, r, pgh[:, 2 * H:3 * H])
                    s = wk.tile([BC, H], F32, tag="s")
                    if gin_sb is not None:
                        nc.vector.tensor_add(s, tmp, gin_sb)
                    else:
                        nc.vector.tensor_add(s, tmp, pginap)
                    n = wk.tile([BC, H], F32, tag="n")
                    nc.scalar.activation(n, s, ACTF.Tanh)
                    pre = wk.tile([BC, H], F32, tag="pre")
                    nc.gpsimd.tensor_mul(pre, z, hb)  # z*h
                    m = wk.tile([BC, H], F32, tag="m")
                    nc.vector.scalar_tensor_tensor(
                        out=m, in0=z, scalar=1.0, in1=n,
                        op0=ALU.subtract, op1=ALU.mult)  # (z-1)*n
                    nc.gpsimd.tensor_sub(hb, pre, m)  # h' = z*h + (1-z)*n
                    for c, w in chh:
                        nc.tensor.transpose(ptr[0:w, c, :], hb[:, c * 128:c * 128 + w], id64)
                        if c % 2 == 0:
                            nc.scalar.copy(out=hT[0:w, c, :], in_=ptr[0:w, c, :])
                        else:
                            nc.vector.tensor_copy(hT[0:w, c, :], ptr[0:w, c, :])
                    if dma_t is not None:
                        nc.sync.dma_start(
                            out=h2t_d[dma_t],
                            in_=hT.rearrange("p c b -> p (c b)"))

                for tau in range(T + 2):
                    do0 = tau < T
                    do1 = 0 <= tau - 1 < T
                    do2 = 0 <= tau - 2 < T
                    # order: cell2, E2, cell1, E1, cell0, E0 so each pgin/pghA
                    # read follows its own writer in program order, while each
                    # cell's EW chain overlaps the next cell's matmuls on PE.
                    if do2:
                        gh_gi_mms(pghB, h2T, h1T, whh2, wih2)
                        ew_cell(pghB, pgin[:, 0:H], None, h2b, h2T, tau - 2)
                    if do1:
                        gh_gi_mms(pghA, h1T, h0T, whh1, wih1)
                        ew_cell(pghA, pgin[:, 0:H], None, h1b, h1T, None)
                    if do0:
                        for lo, w in NCH:
                            for c in range(KC):
                                _mmr(nc, pghA[:, lo:lo + w], h0T[:, c, :],
                                     whh0[:, c, lo:lo + w], c == 0,
                                     (lo == 1024 and c == KC - 1))
                        for lo, w in GICH_RZ:
                            _mmr(nc, pghA[:, lo:lo + w], id64r, gi0[:, lo:lo + w],
                                 False, True)
                        ew_cell(pghA, None, gi0n, h0b, h0T, None)

            # ================= epilogue: fc2 + softmax =================
            with tc.tile_pool(name="fpsum", bufs=2, space="PSUM") as fp, \
                 tc.tile_pool(name="fwk", bufs=3) as fw:
                for t in range(T):
                    ht = fw.tile([128, KC, BC], F32R, tag="ht")
                    nc.sync.dma_start(
                        out=ht.rearrange("p c b -> p (c b)"), in_=h2t_d[t])
                    pf = fp.tile([BC, CP], F32, tag="pf")
                    for c in range(KC):
                        _mmr(nc, pf, ht[:, c, :], w2s[:, c, :], c == 0, c == KC - 1)
                    e = fw.tile([BC, D_CHAR], F32, tag="e")
                    nc.scalar.activation(e, pf[:, 0:D_CHAR], ACTF.Exp)
                    ssum = fw.tile([BC, 1], F32, tag="ssum")
                    nc.vector.reduce_sum(ssum, e, axis=AX.X)
                    rcp = fw.tile([BC, 1], F32, tag="rcp")
                    nc.vector.reciprocal(rcp, ssum)
                    pb = fw.tile([BC, D_CHAR], F32, tag="pb")
                    nc.vector.tensor_scalar_mul(pb, in0=e, scalar1=rcp)
                    nc.sync.dma_start(out=probs[:, t, :], in_=pb)

    nc.compile()
    return nc


def _prep_rec(w, b, kc, ones_row):
    """weight [Gout, Kin] + bias -> [128, kc, Gout_padded] with bias on ones_row."""
    gout, kin = w.shape
    gpad = gout + (gout % 2)
    arr = np.zeros((128, kc, gpad), dtype=np.float32)
    wt = np.ascontiguousarray(w.T)  # [Kin, Gout]
    for c in range(kc):
        lo = c * 128
        hi = min(lo + 128, kin)
        if hi > lo:
            arr[0:hi - lo, c, 0:gout] = wt[lo:hi]
    c, p = divmod(ones_row, 128)
    arr[p, c, 0:gout] = b
    return arr


def kernel(**inputs):
    inputs = {k: np.asarray(v, dtype=np.float32) for k, v in inputs.items()}
    if "nc" not in _CACHE:
        _CACHE["nc"] = build_bass()
    nc = _CACHE["nc"]

    ht_init = np.zeros((128, KC, BC), dtype=np.float32)
    ht_init[117, 3, :] = 1.0
    xt_init = np.zeros((128, KCX, BC), dtype=np.float32)
    xt_init[36, 2, :] = 1.0
    shared = {
        "id64_in": np.eye(64, dtype=np.float32),
        "ht_init": ht_init,
        "xt_init": xt_init,
        "w1s": _prep_rec(inputs["fc1_w"], inputs["fc1_b"], KCX, ONES_ROW_X),
        "wih0s": _prep_rec(SELU_L * inputs["w_ih0"], inputs["b_ih0"], KCX, ONES_ROW_X),
        "whh0s": _prep_rec(inputs["w_hh0"], inputs["b_hh0"], KC, ONES_ROW),
        "wih1s": _prep_rec(inputs["w_ih1"], inputs["b_ih1"], KC, ONES_ROW),
        "whh1s": _prep_rec(inputs["w_hh1"], inputs["b_hh1"], KC, ONES_ROW),
        "wih2s": _prep_rec(inputs["w_ih2"], inputs["b_ih2"], KC, ONES_ROW),
        "whh2s": _prep_rec(inputs["w_hh2"], inputs["b_hh2"], KC, ONES_ROW),
        "w2s": _prep_rec(inputs["fc2_w"], inputs["fc2_b"], KC, ONES_ROW),
    }
    in_maps = []
    for i in range(NCORES):
        m = dict(shared)
        m["z_in"] = np.ascontiguousarray(inputs["z"][i * BC:(i + 1) * BC])
        in_maps.append(m)

    res = bass_utils.run_bass_kernel_spmd(nc, in_maps, list(range(NCORES)))
    global _LAST_RES
    _LAST_RES = res
    out = np.concatenate([r["probs"] for r in res.results], axis=0)
    return out


if __name__ == "__main__":
    np.random.seed(0)
    pass

